# revision 1
# baseline (speedup 1.0000x reference)
"""Trainium2 Bass kernel for nn_BaseTransformer (ensemble member-attention block).

Sharding: data-parallel over batch B=8 across 8 NeuronCores (1 batch each).
Weights/constants replicated. No collectives.

Reference math (per batch b, x = in_tensor[b] as [K=16, C=64, S=4096]):
  value = einsum('ics,oc->ios', x, Wv)
  key   = selu(einsum(x, Wk)); query = selu(einsum(x, Wq))
  gram[c,i,j] = sum_s key[i,c,s] query[j,c,s] / 64        (then * lambda^2 fold)
  A = softmax(gram, axis=i) + I
  transformed[j] = sum_i (A[c,i,j] - 1/16) value_i        (exact mean fold)
  out = selu(x + einsum(transformed, w_out) + b_out)

Layout/dtype scheme (v2):
  - x_bf16 resident as 8 pair tiles [128, S] (members t, t+8); feeds the
    transposed k/q conv (x chunks as PE stationary operand -> k,q come out
    [s, heads]) and the value conv. x_fp32 is re-streamed from HBM in phase 2
    only for the exact residual add.
  - stride-8 head groups (head c = 8u+g) so gram operands are single-stride
    APs and the value gather/scatter DMAs use contiguous partition runs
    (sigma = bit-swap permutation folded into Wv columns / Wout rows).
  - selu(t) = min(alpha*e^t - alpha, relu(t)) composed exactly from
    ACT Exp (bias ln a), ACT Relu, DVE scalar_tensor_tensor (sub/min).
  - mix matmul is block-diagonal over 8 heads x 16 members with the
    B matrices assembled via permutation matmuls (P^T (softmax masked)^T P').
"""

import sys

if "/opt/trn_rl_repo" not in sys.path:
    sys.path.insert(0, "/opt/trn_rl_repo")

import numpy as np

import concourse.bass as bass
import concourse.bacc as bacc
import concourse.mybir as mybir
import concourse.tile as tile

F32 = mybir.dt.float32
BF16 = mybir.dt.bfloat16

K, C, HEADS, S = 16, 64, 64, 4096
NG = 8           # head groups of 8 (stride-8: group g = heads {8u+g})
SC1 = 128        # phase-1 spatial chunk (gram contraction tile)
NCH1 = S // SC1  # 32
SC2 = 512        # phase-2 spatial chunk
NCH2 = S // SC2  # 8

ALPHA = 1.6732632423543772
LAMBDA = 1.0507009873554805
LN_ALPHA = float(np.log(ALPHA))
LN_LAMBDA_ALPHA = float(np.log(LAMBDA * ALPHA))
GRAM_SCALE = float(LAMBDA * LAMBDA / 64.0)


def _pi(u, i):
    return 64 * (i // 8) + 8 * u + (i % 8)


def host_constants(w_value, w_key, w_query, w_out, b_out):
    """Build all replicated device inputs on the host."""
    consts = {}
    # sigma: head c = 8u+g  <->  storage position 8g+u (group-contiguous).
    sigma = np.zeros(64, np.int64)
    for u in range(8):
        for g in range(8):
            sigma[8 * g + u] = 8 * u + g
    wvT = np.ascontiguousarray(w_value.T[:, sigma])
    consts["wvT"] = np.concatenate([wvT, wvT], axis=0).astype(np.float32)
    wkqT = np.ascontiguousarray(np.concatenate([w_key.T, w_query.T], axis=1))
    consts["wkqT"] = np.concatenate([wkqT, wkqT], axis=0).astype(np.float32)
    woutT = np.ascontiguousarray(w_out.T[sigma, :])
    consts["woutT"] = np.concatenate([woutT, woutT], axis=0).astype(np.float32)

    # Gram psum layout: partition = 8j+u (q side), free = 8i+u' (k side).
    # MASK zeroes cross-head entries (u != u').
    mask = np.zeros((128, 128), np.float32)
    for p in range(128):
        for f in range(128):
            if p % 8 == f % 8:
                mask[p, f] = 1.0
    consts["maskg"] = mask

    # P (mm2 lhsT): rows r=(i,u)=8i+u -> out partition pi(u, i); same matrix
    # serves as P' (mm1 rhs) for the j side.
    P = np.zeros((128, 128), np.float32)
    for u in range(8):
        for i in range(16):
            P[8 * i + u, _pi(u, i)] = 1.0
    consts["permP"] = P
    consts["permPp"] = P.copy()

    # DPAT in permuted coords: D[pi(u,i), pi(u,j)] = delta(i,j) - 1/16.
    D = np.zeros((128, 128), np.float32)
    for u in range(8):
        for i in range(16):
            for j in range(16):
                D[_pi(u, i), _pi(u, j)] = (1.0 if i == j else 0.0) - 1.0 / 16.0
    consts["dpat"] = D

    consts["b_out_col"] = np.concatenate([b_out, b_out]).astype(
        np.float32).reshape(128, 1)
    return consts


def build_nc():
    """Build the single-core Bass program (same NEFF on all 8 cores)."""
    nc = bacc.Bacc("TRN2", target_bir_lowering=False, debug=False)

    x_d = nc.dram_tensor("x", [K, C, S], F32, kind="ExternalInput")
    wvT_d = nc.dram_tensor("wvT", [128, 64], F32, kind="ExternalInput")
    wkqT_d = nc.dram_tensor("wkqT", [128, 128], F32, kind="ExternalInput")
    woutT_d = nc.dram_tensor("woutT", [128, 64], F32, kind="ExternalInput")
    mask_d = nc.dram_tensor("maskg", [128, 128], F32, kind="ExternalInput")
    permP_d = nc.dram_tensor("permP", [128, 128], F32, kind="ExternalInput")
    permPp_d = nc.dram_tensor("permPp", [128, 128], F32, kind="ExternalInput")
    dpat_d = nc.dram_tensor("dpat", [128, 128], F32, kind="ExternalInput")
    bo_d = nc.dram_tensor("b_out_col", [128, 1], F32, kind="ExternalInput")
    out_d = nc.dram_tensor("out", [K, C, S], F32, kind="ExternalOutput")

    with tile.TileContext(nc) as tc:
        with (
            tc.tile_pool(name="persist", bufs=1) as persist,
            tc.tile_pool(name="xpool", bufs=1) as xpool,
        ):
            # ---- weights / constants to SBUF (+ bf16 casts) ----
            wv_f = persist.tile([128, 64], F32, tag="wvf")
            nc.sync.dma_start(out=wv_f, in_=wvT_d[:, :])
            wv_sb = persist.tile([128, 64], BF16, tag="wv")
            nc.gpsimd.tensor_copy(wv_sb, wv_f)
            wkq_f = persist.tile([128, 128], F32, tag="wkqf")
            nc.sync.dma_start(out=wkq_f, in_=wkqT_d[:, :])
            wkq_sb = persist.tile([128, 128], BF16, tag="wkq")
            nc.gpsimd.tensor_copy(wkq_sb, wkq_f)
            wo_f = persist.tile([128, 64], F32, tag="wof")
            nc.sync.dma_start(out=wo_f, in_=woutT_d[:, :])
            wo_sb = persist.tile([128, 64], BF16, tag="wo")
            nc.gpsimd.tensor_copy(wo_sb, wo_f)
            mask_sb = persist.tile([128, 128], F32, tag="mask")
            nc.sync.dma_start(out=mask_sb, in_=mask_d[:, :])
            permP_sb = persist.tile([128, 128], F32, tag="permP")
            nc.sync.dma_start(out=permP_sb, in_=permP_d[:, :])
            permPp_sb = persist.tile([128, 128], F32, tag="permPp")
            nc.sync.dma_start(out=permPp_sb, in_=permPp_d[:, :])
            dpat_sb = persist.tile([128, 128], F32, tag="dpat")
            nc.sync.dma_start(out=dpat_sb, in_=dpat_d[:, :])
            bo_sb = persist.tile([128, 1], F32, tag="bo")
            nc.sync.dma_start(out=bo_sb, in_=bo_d[:, :])
            lna_sb = persist.tile([128, 1], F32, tag="lna")
            nc.vector.memset(lna_sb, LN_ALPHA)
            lnla_sb = persist.tile([128, 1], F32, tag="lnla")
            nc.vector.memset(lnla_sb, LN_LAMBDA_ALPHA)
            zero_sb = persist.tile([128, 1], F32, tag="zero")
            nc.vector.memset(zero_sb, 0.0)

            # ---- x: stream fp32, cast to resident bf16 pair tiles ----
            x_sb = []
            with tc.tile_pool(name="xload", bufs=3) as xload:
                for t in range(8):
                    xf = xload.tile([128, S], F32, tag="xf")
                    nc.sync.dma_start(out=xf[0:64, :], in_=x_d[t, :, :])
                    nc.sync.dma_start(out=xf[64:128, :], in_=x_d[t + 8, :, :])
                    xb = xpool.tile([128, S], BF16, tag=f"x{t}")
                    if t % 3 == 1:
                        nc.scalar.copy(xb, xf)
                    elif t % 3 == 2:
                        nc.gpsimd.tensor_copy(xb, xf)
                    else:
                        nc.vector.tensor_copy(xb, xf)
                    x_sb.append(xb)

            # BigB result tiles (persist into phase 2), bf16 for the mix matmul
            bigB = []
            for g in range(NG):
                bigB_t = persist.tile([128, 128], BF16, tag=f"bigB{g}")
                bigB.append(bigB_t)

            # =========================== PHASE 1 ===========================
            with (
                tc.tile_pool(name="p1sb", bufs=3) as p1sb,
                tc.tile_pool(name="p1sc", bufs=3) as p1sc,
                tc.tile_pool(name="kqps", bufs=2, space="PSUM") as kqps,
                tc.tile_pool(name="gramps", bufs=1, space="PSUM") as gramps,
            ):
                gram_ps = []
                for gb in range(2):
                    gram_t = gramps.tile([128, 512], F32, tag=f"gram{gb}")
                    gram_ps.append(gram_t)

                for sc in range(NCH1):
                    sl = slice(SC1 * sc, SC1 * (sc + 1))
                    # kqT free layout: half*1024 + m*64 + c   (bf16)
                    kqT = p1sb.tile([128, K * 128], BF16, tag="kqT")
                    for blk in range(2):  # member blocks [0..8), [8..16)
                        ps = kqps.tile([128, 8 * 128], F32, tag="kqps")
                        for mb in range(8):
                            m = blk * 8 + mb
                            xt = x_sb[m % 8]
                            rhalf = slice(0, 64) if m < 8 else slice(64, 128)
                            nc.tensor.matmul(
                                ps[:, 128 * mb: 128 * (mb + 1)],
                                xt[rhalf, sl], wkq_sb[rhalf, :],
                                start=True, stop=True,
                            )
                        # selu: e2 = exp(kq + ln a); r = relu(kq);
                        # out = (e2 - a) min r   (all bf16 outputs).
                        # e2/r stored half-split (h, mb, c) so the stt reads
                        # contiguous halves (DVE 2x bf16 mode).
                        e2 = p1sc.tile([128, 8 * 128], BF16, tag="e2")
                        e2v = e2.rearrange("p (h mb c) -> p mb h c",
                                           mb=8, h=2, c=64)
                        nc.scalar.activation(
                            out=e2v, in_=ps,
                            func=mybir.ActivationFunctionType.Exp,
                            bias=lna_sb[:, 0:1])
                        r = p1sc.tile([128, 8 * 128], BF16, tag="r")
                        rv = r.rearrange("p (h mb c) -> p mb h c",
                                         mb=8, h=2, c=64)
                        if (sc + blk) % 2 == 0:
                            nc.scalar.activation(
                                out=rv, in_=ps,
                                func=mybir.ActivationFunctionType.Relu,
                                bias=zero_sb[:, 0:1])
                        else:
                            nc.vector.tensor_scalar(
                                out=rv, in0=ps, scalar1=0.0, scalar2=None,
                                op0=mybir.AluOpType.max)
                        for half in range(2):
                            nc.vector.scalar_tensor_tensor(
                                out=kqT[:, 1024 * half + 512 * blk:
                                        1024 * half + 512 * (blk + 1)],
                                in0=e2[:, 512 * half: 512 * (half + 1)],
                                scalar=ALPHA,
                                in1=r[:, 512 * half: 512 * (half + 1)],
                                op0=mybir.AluOpType.subtract,
                                op1=mybir.AluOpType.min)
                    # gram: lhsT = q side (M = 8j+u), rhs = k side (N = 8i+u'),
                    # single-stride [[8,128]] APs at offset g
                    vq = kqT.rearrange("p (f e) -> p e f", f=256, e=8)
                    for g in range(NG):
                        q_ap = vq[:, g, 128:256]
                        k_ap = vq[:, g, 0:128]
                        nc.tensor.matmul(
                            gram_ps[g // 4][:, 128 * (g % 4): 128 * (g % 4 + 1)],
                            q_ap, k_ap,
                            start=(sc == 0 and g % 4 == 0),
                            stop=(sc == NCH1 - 1 and g % 4 == 3))

                # ---- softmax (no max-sub; range pre-verified) + BigB ----
                for g in range(NG):
                    gp = gram_ps[g // 4][:, 128 * (g % 4): 128 * (g % 4 + 1)]
                    E = p1sc.tile([128, 128], F32, tag="E")
                    nc.scalar.activation(
                        out=E, in_=gp,
                        func=mybir.ActivationFunctionType.Exp,
                        bias=zero_sb[:, 0:1], scale=GRAM_SCALE)
                    Ssum = p1sc.tile([128, 8], F32, tag="Ssum")
                    nc.vector.tensor_reduce(
                        out=Ssum,
                        in_=E.rearrange("p (i u) -> p u i", i=16, u=8),
                        axis=mybir.AxisListType.X, op=mybir.AluOpType.add)
                    R = p1sc.tile([128, 8], F32, tag="R")
                    nc.vector.reciprocal(out=R, in_=Ssum)
                    Eu = E.rearrange("p (i u) -> p u i", i=16, u=8)
                    for u in range(8):
                        nc.vector.tensor_scalar(
                            out=Eu[:, u, :], in0=Eu[:, u, :],
                            scalar1=R[:, u: u + 1], scalar2=None,
                            op0=mybir.AluOpType.mult)
                    nc.vector.tensor_tensor(
                        out=E, in0=E, in1=mask_sb, op=mybir.AluOpType.mult)
                    c_ps = kqps.tile([128, 128], F32, tag="kqps")
                    nc.tensor.matmul(c_ps, E, permPp_sb, start=True, stop=True)
                    c_sb = p1sc.tile([128, 128], F32, tag="permcsb")
                    nc.scalar.copy(c_sb, c_ps)
                    b_ps = kqps.tile([128, 128], F32, tag="kqps")
                    nc.tensor.matmul(b_ps, permP_sb, c_sb, start=True, stop=True)
                    nc.vector.scalar_tensor_tensor(
                        out=bigB[g], in0=b_ps, scalar=1.0, in1=dpat_sb,
                        op0=mybir.AluOpType.mult, op1=mybir.AluOpType.add)

            # =========================== PHASE 2 ===========================
            # SBUF->SBUF DMAs lower to SP-serial DIRECT2D (slow), so the
            # partition-regrouping shuffles round-trip through DRAM scratch:
            # DRAM-involved DMAs ride the fast DGE path and DRAM-side access
            # patterns may stride arbitrarily. Scratch is per-chunk (no WAR).
            # Software-pipelined 3 stages: value(pc) | mix(pc-1) | out(pc-2).
            # vscr[pc][g][64*i2 + 8u + it][s] -- gather g is a plain 2D load.
            # mscr[pc][jt][64*j2 + 8g + u][s] -- tload jt is a plain 2D load.
            vscr_d = nc.dram_tensor("vscr", [NCH2, NG, 128, SC2], BF16)
            mscr_d = nc.dram_tensor("mscr", [NCH2, 8, 128, SC2], BF16)
            with (
                tc.tile_pool(name="xsp", bufs=4) as xsp,
                tc.tile_pool(name="p2sc", bufs=6) as p2sc,
                tc.tile_pool(name="p2out", bufs=3) as p2outp,
                tc.tile_pool(name="vps", bufs=3, space="PSUM") as vps,
                tc.tile_pool(name="mps", bufs=3, space="PSUM") as mps,
                tc.tile_pool(name="ops", bufs=2, space="PSUM") as ops,
            ):
                xv_d = x_d.rearrange("(m2 mt) c s -> mt m2 c s", m2=2, mt=8)
                ov_d = out_d.rearrange("(m2 mt) c s -> mt m2 c s", m2=2, mt=8)
                vstores = {}
                mstores = {}

                def stage_value(pc):
                    sl = slice(SC2 * pc, SC2 * (pc + 1))
                    vdst = vscr_d[pc].rearrange(
                        "g (i2 u it) s -> it i2 g u s", i2=2, u=8, it=8)
                    stores = []
                    for t in range(8):
                        ps = vps.tile([128, SC2], F32, tag="vps")
                        nc.tensor.matmul(
                            ps[0:64, :], wv_sb[0:64, :], x_sb[t][0:64, sl],
                            start=True, stop=True)
                        nc.tensor.matmul(
                            ps[64:128, :], wv_sb[64:128, :], x_sb[t][64:128, sl],
                            start=True, stop=True)
                        vpair = p2sc.tile([128, SC2], BF16, tag="vpair")
                        nc.vector.tensor_copy(vpair, ps)
                        for i2 in range(2):
                            sti = nc.sync.dma_start(
                                out=vdst[t, i2],
                                in_=vpair[64 * i2: 64 * (i2 + 1), :])
                            stores.append(sti)
                    vstores[pc] = stores

                def stage_mix(pc):
                    stores = vstores.pop(pc)
                    mdst = mscr_d[pc].rearrange(
                        "jt (j2 gg u) s -> gg j2 u jt s", j2=2, gg=8, u=8)
                    mst = []
                    for g in range(NG):
                        pm = mps.tile([128, SC2], F32, tag="mps")
                        vg = p2sc.tile([128, SC2], BF16, tag="vg")
                        gi = nc.scalar.dma_start(out=vg, in_=vscr_d[pc, g])
                        for sti in stores:
                            tile.add_dep_helper(
                                gi.ins, sti.ins, reason="gather after vstores")
                        nc.tensor.matmul(pm, bigB[g], vg, start=True, stop=True)
                        mg = p2sc.tile([128, SC2], BF16, tag="mg")
                        nc.scalar.copy(mg, pm)
                        for j2 in range(2):
                            si = nc.sync.dma_start(
                                out=mdst[g, j2],
                                in_=mg[64 * j2: 64 * (j2 + 1), :])
                            mst.append(si)
                    mstores[pc] = mst

                def stage_out(pc):
                    sl = slice(SC2 * pc, SC2 * (pc + 1))
                    mst = mstores.pop(pc)
                    for jt in range(8):
                        xs = xsp.tile([128, SC2], F32, tag="xs")
                        nc.sync.dma_start(out=xs, in_=xv_d[jt, :, :, sl])
                        tpair = p2sc.tile([128, SC2], BF16, tag="tpair")
                        li = nc.scalar.dma_start(out=tpair, in_=mscr_d[pc, jt])
                        for si in mst:
                            tile.add_dep_helper(
                                li.ins, si.ins, reason="tload after mstores")
                        po = ops.tile([128, SC2], F32, tag="ops")
                        nc.tensor.matmul(
                            po[0:64, :], wo_sb[0:64, :], tpair[0:64, :],
                            start=True, stop=True)
                        nc.tensor.matmul(
                            po[64:128, :], wo_sb[64:128, :], tpair[64:128, :],
                            start=True, stop=True)
                        # y = po + b_out + x   (exact fp32 residual)
                        ty = p2sc.tile([128, SC2], F32, tag="ty")
                        nc.vector.scalar_tensor_tensor(
                            out=ty, in0=po, scalar=bo_sb[:, 0:1], in1=xs,
                            op0=mybir.AluOpType.add, op1=mybir.AluOpType.add)
                        # selu(y) = min(l*a*e^y - l*a, l*relu(y))
                        e2f = p2sc.tile([128, SC2], F32, tag="fe2")
                        nc.scalar.activation(
                            out=e2f, in_=ty,
                            func=mybir.ActivationFunctionType.Exp,
                            bias=lnla_sb[:, 0:1])
                        r2f = p2sc.tile([128, SC2], F32, tag="fr2")
                        nc.vector.tensor_scalar(
                            out=r2f, in0=ty, scalar1=0.0, scalar2=LAMBDA,
                            op0=mybir.AluOpType.max, op1=mybir.AluOpType.mult)
                        o_sb = p2outp.tile([128, SC2], F32, tag="osb")
                        nc.vector.scalar_tensor_tensor(
                            out=o_sb, in0=e2f, scalar=float(LAMBDA * ALPHA),
                            in1=r2f,
                            op0=mybir.AluOpType.subtract,
                            op1=mybir.AluOpType.min)
                        nc.sync.dma_start(out=ov_d[jt, :, :, sl], in_=o_sb)

                for pc in range(NCH2 + 2):
                    if pc < NCH2:
                        stage_value(pc)
                    if 1 <= pc <= NCH2:
                        stage_mix(pc - 1)
                    if pc >= 2:
                        stage_out(pc - 2)
    nc.compile()
    return nc


_NC_CACHE = None


def _get_nc():
    global _NC_CACHE
    if _NC_CACHE is None:
        _NC_CACHE = build_nc()
    return _NC_CACHE


def kernel(in_tensor, w_value, w_key, w_query, w_out, b_out, **_ignored):
    in_tensor = np.asarray(in_tensor, dtype=np.float32)
    w_value = np.asarray(w_value, dtype=np.float32)
    w_key = np.asarray(w_key, dtype=np.float32)
    w_query = np.asarray(w_query, dtype=np.float32)
    w_out = np.asarray(w_out, dtype=np.float32)
    b_out = np.asarray(b_out, dtype=np.float32)

    B = in_tensor.shape[0]
    assert B == 8
    consts = host_constants(w_value, w_key, w_query, w_out, b_out)

    nc = _get_nc()
    in_maps = []
    for b in range(B):
        m = {"x": np.ascontiguousarray(in_tensor[b].reshape(K, C, S))}
        m.update(consts)
        in_maps.append(m)

    from concourse.bass_utils import run_bass_kernel_spmd

    res = run_bass_kernel_spmd(nc, in_maps, core_ids=list(range(8)))
    outs = [res.results[b]["out"].reshape(K, C, 64, 64) for b in range(B)]
    return np.stack(outs, axis=0).astype(np.float32)


if __name__ == "__main__":
    build_nc()
    print("built ok")



# revision 16
# speedup vs baseline: 3.0473x; 3.0473x over previous
"""Trainium2 Bass kernel for nn_BaseTransformer (ensemble member-attention).

Sharding: data-parallel over batch B=8 across 8 NeuronCores (1 batch each).

v3 design (DMA-dispatch-count diet vs v2 baseline):
  - Host uploads x pre-packed as bf16 pair tiles xb[t] = members (2t, 2t+1)
    rows (m2*64 + c); pure reshape + cast on host. Device never casts x.
  - Value conv first (block-diag WvT, full 128-contract), v tiles scattered
    to DRAM vscr in group-major layout with 4 KB-run descriptors; the
    gathers ride during phase 1 (kq conv + gram), fully hidden.
  - kq conv: x chunk stationary, streams block-diag [128,256] wkq ->
    psum col order (tq, m2, h, o) == kqT col m*128 + h*64 + o for member
    m = 2t + m2, so selu (exp/relu/stt) writes are plain contiguous and
    the gram operands are clean 3-dim APs [s | m:128 | u:8] @ h*64+g.
  - softmax -> bigB via host permutation matmuls (P, P', dpat) with
    pi(u, m) = 64*(m%2) + 8u + m//2.
  - mix per head-group g (bigB stationary), tg scattered to tscr
    (pair-major), tp gathered, out conv = wo2 matmul + residual
    lam~*I matmul (lam~ = bf16(lambda)), selu via exp/ts/stt, out
    written as bf16 [8,128,4096]; host unpacks (reshape + fp32 cast).
  - Total ~56 big DMAs (vs ~540 small in v2), spread over SP + POOL
    SWDGE queues.
"""

import sys

if "/opt/trn_rl_repo" not in sys.path:
    sys.path.insert(0, "/opt/trn_rl_repo")

import numpy as np

import concourse.bass as bass
import concourse.bacc as bacc
import concourse.mybir as mybir
import concourse.tile as tile

F32 = mybir.dt.float32
BF16 = mybir.dt.bfloat16

K, C, HEADS, S = 16, 64, 64, 4096
NG = 8
SC1 = 128          # phase-1 s-chunk (gram contraction tile)
NCH1 = S // SC1    # 32
SC2 = 512          # phase-2 s-chunk (one psum bank)
NCH2 = S // SC2    # 8

ALPHA = 1.6732632423543772
LAMBDA = 1.0507009873554805
LN_ALPHA = float(np.log(ALPHA))
LN_LAMBDA_ALPHA = float(np.log(LAMBDA * ALPHA))
GRAM_SCALE = float(LAMBDA * LAMBDA / 64.0)
LAM_BF16 = 1.046875  # bf16(lambda); residual uses this exactly


def _sigma(p):
    # storage head position p = 8g+u holds real head 8u+g
    return 8 * (p % 8) + (p // 8)


def _pi(u, m):
    # vg/tg row for (sub-head u, member m)
    return 64 * (m % 2) + 8 * u + (m // 2)


def host_constants(w_value, w_key, w_query, w_out, b_out):
    consts = {}
    # kq conv rhs [128, 256]: [(m2, c), (h, m2', o)] = delta(m2,m2')*W_h[o,c]
    wkq2 = np.zeros((128, 256), np.float32)
    for m2 in range(2):
        rows = slice(m2 * 64, (m2 + 1) * 64)
        wkq2[rows, m2 * 64: m2 * 64 + 64] = w_key.T
        wkq2[rows, 128 + m2 * 64: 128 + m2 * 64 + 64] = w_query.T
    consts["wkq2"] = wkq2

    # value conv lhsT [128,128]: [(m2,c),(a,p)] = delta(m2,a)*Wv[sigma(p),c]
    wv2 = np.zeros((128, 128), np.float32)
    for a in range(2):
        for p in range(64):
            wv2[a * 64:(a + 1) * 64, a * 64 + p] = w_value[_sigma(p), :]
    consts["wv2"] = wv2

    # out conv lhsT [128,128]: [(a,p'),(a',o)] = delta(a,a')*lam*Wout[o,sig(p')]
    wo2 = np.zeros((128, 128), np.float32)
    for a in range(2):
        for p in range(64):
            wo2[a * 64 + p, a * 64:(a + 1) * 64] = (
                LAMBDA * w_out[:, _sigma(p)])
    consts["wo2"] = wo2

    # residual lhsT: lam~ * I (entries exactly representable in bf16)
    consts["resI"] = (LAM_BF16 * np.eye(128)).astype(np.float32)

    # gram psum: partition (8j+u), free (8i+u'); mask kills u != u'
    mask = np.zeros((128, 128), np.float32)
    for p in range(128):
        for f in range(128):
            if p % 8 == f % 8:
                mask[p, f] = 1.0
    consts["maskg"] = mask

    # P[(8m+u), pi(u,m)] = 1  (serves both sides)
    P = np.zeros((128, 128), np.float32)
    for u in range(8):
        for m in range(16):
            P[8 * m + u, _pi(u, m)] = 1.0
    consts["permP"] = P
    consts["permPp"] = P.copy()

    # dpat[pi(u,i), pi(u,j)] = delta(i,j) - 1/16
    D = np.zeros((128, 128), np.float32)
    for u in range(8):
        for i in range(16):
            for j in range(16):
                D[_pi(u, i), _pi(u, j)] = (1.0 if i == j else 0.0) - 1.0 / 16.0
    consts["dpat"] = D

    bo2 = np.concatenate([b_out, b_out]).astype(np.float32)
    consts["be_col"] = (bo2 + LN_LAMBDA_ALPHA).reshape(128, 1)
    consts["br_col"] = (LAMBDA * bo2).reshape(128, 1)
    return consts


def make_in_maps(in_tensor, consts):
    """Per-core input dicts. in_tensor fp32 [8,16,64,64,64]."""
    import ml_dtypes
    in_maps = []
    for b in range(8):
        xb = np.ascontiguousarray(
            in_tensor[b].reshape(8, 128, S)).astype(ml_dtypes.bfloat16)
        m = {"xb": xb}
        m.update(consts)
        in_maps.append(m)
    return in_maps


def build_nc():
    nc = bacc.Bacc("TRN2", target_bir_lowering=False, debug=False)

    xb_d = nc.dram_tensor("xb", [8, 128, S], BF16, kind="ExternalInput")
    wkq2_d = nc.dram_tensor("wkq2", [128, 256], F32, kind="ExternalInput")
    wv2_d = nc.dram_tensor("wv2", [128, 128], F32, kind="ExternalInput")
    wo2_d = nc.dram_tensor("wo2", [128, 128], F32, kind="ExternalInput")
    resI_d = nc.dram_tensor("resI", [128, 128], F32, kind="ExternalInput")
    mask_d = nc.dram_tensor("maskg", [128, 128], F32, kind="ExternalInput")
    permP_d = nc.dram_tensor("permP", [128, 128], F32, kind="ExternalInput")
    permPp_d = nc.dram_tensor("permPp", [128, 128], F32, kind="ExternalInput")
    dpat_d = nc.dram_tensor("dpat", [128, 128], F32, kind="ExternalInput")
    be_d = nc.dram_tensor("be_col", [128, 1], F32, kind="ExternalInput")
    br_d = nc.dram_tensor("br_col", [128, 1], F32, kind="ExternalInput")
    out_d = nc.dram_tensor("out", [8, 128, S], BF16, kind="ExternalOutput")

    vscr_d = nc.dram_tensor("vscr", [NG, 128, S], BF16)
    tscr_d = nc.dram_tensor("tscr", [8, 128, S], BF16)

    with tile.TileContext(nc) as tc:
        with (
            tc.tile_pool(name="persist", bufs=1) as persist,
            tc.tile_pool(name="xpool", bufs=1) as xpool,
            tc.tile_pool(name="sc8k", bufs=1) as sc8k,
            tc.tile_pool(name="outp", bufs=3) as outp,
        ):
            # ---- constants ----
            def load_cast(dram, shape, tag, dtype=BF16, eng=None):
                f = persist.tile(shape, F32, tag=tag + "f")
                nc.sync.dma_start(out=f, in_=dram[:, :])
                if dtype == F32:
                    return f
                b = persist.tile(shape, dtype, tag=tag)
                (eng or nc.gpsimd).tensor_copy(b, f)
                return b

            wkq_sb = load_cast(wkq2_d, [128, 256], "wkq")
            wv_sb = load_cast(wv2_d, [128, 128], "wv")
            wo_sb = load_cast(wo2_d, [128, 128], "wo")
            resI_sb = load_cast(resI_d, [128, 128], "resI")
            mask_sb = load_cast(mask_d, [128, 128], "mask", F32)
            permP_sb = load_cast(permP_d, [128, 128], "permP", F32)
            permPp_sb = load_cast(permPp_d, [128, 128], "permPp", F32)
            dpat_sb = load_cast(dpat_d, [128, 128], "dpat", F32)
            be_sb = persist.tile([128, 1], F32, tag="be")
            nc.sync.dma_start(out=be_sb, in_=be_d[:, :])
            br_sb = persist.tile([128, 1], F32, tag="br")
            nc.sync.dma_start(out=br_sb, in_=br_d[:, :])
            lna_sb = persist.tile([128, 1], F32, tag="lna")
            nc.vector.memset(lna_sb, LN_ALPHA)
            zero_sb = persist.tile([128, 1], F32, tag="zero")
            nc.vector.memset(zero_sb, 0.0)

            # ---- x tiles (already bf16 in DRAM) ----
            x_sb = []
            for t in range(8):
                xt = xpool.tile([128, S], BF16, tag=f"x{t}")
                nc.sync.dma_start(out=xt, in_=xb_d[t])
                x_sb.append(xt)

            # ---- value conv + scatter (overlaps phase 1) ----
            vstores = []
            # vscr[g] row = 64a + 8u + b holds member 2b+a, head 8u+g;
            # scatter of tile t: src partition (a, g, u) -> [b=t][a, g, u, s]
            vsc_view = vscr_d.rearrange(
                "g (a u b) s -> b a g u s", a=2, u=8, b=8)
            with tc.tile_pool(name="vps", bufs=3, space="PSUM") as vps:
                for t in range(8):
                    vt = sc8k.tile([128, S], BF16, tag=f"s{t}")
                    for ch in range(NCH2):
                        sl = slice(SC2 * ch, SC2 * (ch + 1))
                        ps = vps.tile([128, SC2], F32, tag="vps")
                        nc.tensor.matmul(ps, wv_sb, x_sb[t][:, sl],
                                         start=True, stop=True)
                        nc.vector.tensor_copy(vt[:, sl], ps)
                    # src partitions (a half): p = 8g + u, g-major ->
                    # dst dims (g, u, s); one DMA per a (3-dim DMA AP limit)
                    for a in range(2):
                        si = nc.gpsimd.dma_start(
                            out=vsc_view[t, a],
                            in_=vt[64 * a: 64 * (a + 1), :])
                        vstores.append(si)

            # vg gathers (reuse v slots; dep on ALL v scatters)
            vg_sb = []
            for g in range(NG):
                vg = sc8k.tile([128, S], BF16, tag=f"s{g}")
                gi = nc.sync.dma_start(out=vg, in_=vscr_d[g])
                for si in vstores:
                    tile.add_dep_helper(gi.ins, si.ins, reason="vg after vsc")
                vg_sb.append(vg)

            # ---- phase 1: kq conv + gram ----
            bigB = []
            for g in range(NG):
                bigB_t = persist.tile([128, 128], BF16, tag=f"bigB{g}")
                bigB.append(bigB_t)

            with (
                tc.tile_pool(name="kqT", bufs=2) as kqTp,
                tc.tile_pool(name="p1sc", bufs=3) as p1sc,
                tc.tile_pool(name="kqps", bufs=2, space="PSUM") as kqps,
                tc.tile_pool(name="gramps", bufs=1, space="PSUM") as gramps,
            ):
                gram_ps = []
                for gb in range(2):
                    gram_t = gramps.tile([128, 512], F32, tag=f"gram{gb}")
                    gram_ps.append(gram_t)

                for sc in range(NCH1):
                    sl = slice(SC1 * sc, SC1 * (sc + 1))
                    # kqT col = h*1024 + m*64 + o, member m = 8q + 2tq + m2
                    kqT = kqTp.tile([128, 2048], BF16, tag="kqT")
                    for q in range(2):
                        # psum col = tq*256 + h*128 + z, z = m2*64 + o
                        ps = kqps.tile([128, 1024], F32, tag="kqps")
                        for tq in range(4):
                            t = q * 4 + tq
                            nc.tensor.matmul(
                                ps[:, 256 * tq: 256 * (tq + 1)],
                                x_sb[t][:, sl], wkq_sb,
                                start=True, stop=True)
                        # selu(t)/lam = min(alpha e^t - alpha, relu(t))
                        # e2/r col = h*512 + tq*128 + z (psum regrouped)
                        psv = ps.rearrange("p (tq h z) -> p h tq z",
                                           tq=4, h=2, z=128)
                        e2 = p1sc.tile([128, 1024], BF16, tag="e2")
                        e2v = e2.rearrange("p (h tq z) -> p h tq z",
                                           tq=4, h=2, z=128)
                        nc.scalar.activation(
                            out=e2v, in_=psv,
                            func=mybir.ActivationFunctionType.Exp,
                            bias=lna_sb[:, 0:1])
                        r = p1sc.tile([128, 1024], BF16, tag="r")
                        rv = r.rearrange("p (h tq z) -> p h tq z",
                                         tq=4, h=2, z=128)
                        if (sc + q) % 2 == 0:
                            nc.scalar.activation(
                                out=rv, in_=psv,
                                func=mybir.ActivationFunctionType.Relu,
                                bias=zero_sb[:, 0:1])
                        else:
                            nc.vector.tensor_scalar(
                                out=rv, in0=psv, scalar1=0.0, scalar2=None,
                                op0=mybir.AluOpType.max)
                        # kqT q-half: col = h*1024 + q*512 + tq*128 + z
                        kqo = kqT.rearrange("p (h q tq z) -> p h tq z q",
                                            h=2, q=2, tq=4, z=128)
                        nc.vector.scalar_tensor_tensor(
                            out=kqo[:, :, :, :, q],
                            in0=e2, scalar=ALPHA, in1=r,
                            op0=mybir.AluOpType.subtract,
                            op1=mybir.AluOpType.min)
                    # gram operand: col = h*1024 + f*8 + e, f = 8m+u, e = g
                    kqv = kqT.rearrange("p (h f e) -> p h e f",
                                        h=2, f=128, e=8)
                    for g in range(NG):
                        q_ap = kqv[:, 1, g, :]
                        k_ap = kqv[:, 0, g, :]
                        nc.tensor.matmul(
                            gram_ps[g // 4][:, 128 * (g % 4): 128 * (g % 4 + 1)],
                            q_ap, k_ap,
                            start=(sc == 0 and g % 4 == 0),
                            stop=(sc == NCH1 - 1 and g % 4 == 3))

                # ---- softmax + bigB ----
                for g in range(NG):
                    gp = gram_ps[g // 4][:, 128 * (g % 4): 128 * (g % 4 + 1)]
                    E = p1sc.tile([128, 128], F32, tag="E")
                    nc.scalar.activation(
                        out=E, in_=gp,
                        func=mybir.ActivationFunctionType.Exp,
                        bias=zero_sb[:, 0:1], scale=GRAM_SCALE)
                    Ssum = p1sc.tile([128, 8], F32, tag="Ssum")
                    nc.vector.tensor_reduce(
                        out=Ssum,
                        in_=E.rearrange("p (i u) -> p u i", i=16, u=8),
                        axis=mybir.AxisListType.X, op=mybir.AluOpType.add)
                    R = p1sc.tile([128, 8], F32, tag="R")
                    nc.vector.reciprocal(out=R, in_=Ssum)
                    Eu = E.rearrange("p (i u) -> p u i", i=16, u=8)
                    for u in range(8):
                        nc.vector.tensor_scalar(
                            out=Eu[:, u, :], in0=Eu[:, u, :],
                            scalar1=R[:, u: u + 1], scalar2=None,
                            op0=mybir.AluOpType.mult)
                    nc.vector.tensor_tensor(
                        out=E, in0=E, in1=mask_sb, op=mybir.AluOpType.mult)
                    c_ps = kqps.tile([128, 1024], F32, tag="kqps")
                    nc.tensor.matmul(c_ps[:, 0:128], E, permPp_sb,
                                     start=True, stop=True)
                    c_sb = p1sc.tile([128, 128], F32, tag="csb")
                    nc.scalar.copy(c_sb, c_ps[:, 0:128])
                    b_ps = kqps.tile([128, 1024], F32, tag="kqps")
                    nc.tensor.matmul(b_ps[:, 0:128], permP_sb, c_sb,
                                     start=True, stop=True)
                    nc.vector.scalar_tensor_tensor(
                        out=bigB[g], in0=b_ps[:, 0:128], scalar=1.0,
                        in1=dpat_sb,
                        op0=mybir.AluOpType.mult, op1=mybir.AluOpType.add)

            # ---- phase 2: mix -> t shuffle -> out conv + selu ----
            tsc_view = tscr_d.rearrange(
                "b (a gg u) s -> gg a u b s", a=2, gg=8, u=8)
            tstores = []
            # mix psum gets all 8 banks: tg's slot-reuse WAR dep (on vg[g]'s
            # last mix read) would deadlock a smaller rotating pool.
            with tc.tile_pool(name="mps", bufs=8, space="PSUM") as mps:
                for g in range(NG):
                    tg = sc8k.tile([128, S], BF16, tag=f"s{g}")
                    for ch in range(NCH2):
                        sl = slice(SC2 * ch, SC2 * (ch + 1))
                        pm = mps.tile([128, SC2], F32, tag="mps")
                        nc.tensor.matmul(pm, bigB[g], vg_sb[g][:, sl],
                                         start=True, stop=True)
                        nc.scalar.copy(tg[:, sl], pm)
                    # src partitions (a half): p = 8u + b, u-major ->
                    # dst dims (u, b, s)
                    for a in range(2):
                        si = nc.gpsimd.dma_start(
                            out=tsc_view[g, a],
                            in_=tg[64 * a: 64 * (a + 1), :])
                        tstores.append(si)

            with (
                tc.tile_pool(name="ops", bufs=3, space="PSUM") as ops,
                tc.tile_pool(name="p2sc", bufs=4) as p2sc,
            ):
                for t in range(8):
                    tp = sc8k.tile([128, S], BF16, tag=f"s{t}")
                    li = nc.sync.dma_start(out=tp, in_=tscr_d[t])
                    for si in tstores:
                        tile.add_dep_helper(li.ins, si.ins,
                                            reason="tp after tsc")
                    ot = outp.tile([128, S], BF16, tag="ot")
                    for ch in range(NCH2):
                        sl = slice(SC2 * ch, SC2 * (ch + 1))
                        po = ops.tile([128, SC2], F32, tag="ops")
                        nc.tensor.matmul(po, wo_sb, tp[:, sl],
                                         start=True, stop=False)
                        nc.tensor.matmul(po, resI_sb, x_sb[t][:, sl],
                                         start=False, stop=True)
                        # psum ~ lam*(x + Wout t); selu:
                        # out = min(lam a e^z - lam a, relu(lam z + lam b))
                        ef = p2sc.tile([128, SC2], BF16, tag="ef")
                        nc.scalar.activation(
                            out=ef, in_=po,
                            func=mybir.ActivationFunctionType.Exp,
                            bias=be_sb[:, 0:1], scale=float(1.0 / LAMBDA))
                        rf = p2sc.tile([128, SC2], BF16, tag="rf")
                        nc.vector.tensor_scalar(
                            out=rf, in0=po, scalar1=br_sb[:, 0:1],
                            scalar2=0.0,
                            op0=mybir.AluOpType.add, op1=mybir.AluOpType.max)
                        nc.vector.scalar_tensor_tensor(
                            out=ot[:, sl], in0=ef,
                            scalar=float(LAMBDA * ALPHA), in1=rf,
                            op0=mybir.AluOpType.subtract,
                            op1=mybir.AluOpType.min)
                    nc.sync.dma_start(out=out_d[t], in_=ot)
    nc.compile()
    return nc


_NC_CACHE = None


def _get_nc():
    global _NC_CACHE
    if _NC_CACHE is None:
        _NC_CACHE = build_nc()
    return _NC_CACHE


def kernel(in_tensor, w_value, w_key, w_query, w_out, b_out, **_ignored):
    in_tensor = np.asarray(in_tensor, dtype=np.float32)
    consts = host_constants(
        np.asarray(w_value, dtype=np.float32),
        np.asarray(w_key, dtype=np.float32),
        np.asarray(w_query, dtype=np.float32),
        np.asarray(w_out, dtype=np.float32),
        np.asarray(b_out, dtype=np.float32))
    assert in_tensor.shape[0] == 8
    in_maps = make_in_maps(in_tensor, consts)

    nc = _get_nc()
    from concourse.bass_utils import run_bass_kernel_spmd
    res = run_bass_kernel_spmd(nc, in_maps, core_ids=list(range(8)))
    outs = [np.asarray(res.results[b]["out"]).astype(np.float32)
            .reshape(K, C, 64, 64) for b in range(8)]
    return np.stack(outs, axis=0)


if __name__ == "__main__":
    build_nc()
    print("built ok")


# revision 38
# speedup vs baseline: 3.0501x; 1.0009x over previous
"""Trainium2 Bass kernel for nn_BaseTransformer (ensemble member-attention).

Sharding: data-parallel over batch B=8 across 8 NeuronCores (1 batch each).

v3 design (DMA-dispatch-count diet vs v2 baseline):
  - Host uploads x pre-packed as bf16 pair tiles xb[t] = members (2t, 2t+1)
    rows (m2*64 + c); pure reshape + cast on host. Device never casts x.
  - Value conv first (block-diag WvT, full 128-contract), v tiles scattered
    to DRAM vscr in group-major layout with 4 KB-run descriptors; the
    gathers ride during phase 1 (kq conv + gram), fully hidden.
  - kq conv: x chunk stationary, streams block-diag [128,256] wkq ->
    psum col order (tq, m2, h, o) == kqT col m*128 + h*64 + o for member
    m = 2t + m2, so selu (exp/relu/stt) writes are plain contiguous and
    the gram operands are clean 3-dim APs [s | m:128 | u:8] @ h*64+g.
  - softmax -> bigB via host permutation matmuls (P, P', dpat) with
    pi(u, m) = 64*(m%2) + 8u + m//2.
  - mix per head-group g (bigB stationary), tg scattered to tscr
    (pair-major), tp gathered, out conv = wo2 matmul + residual
    lam~*I matmul (lam~ = bf16(lambda)), selu via exp/ts/stt, out
    written as bf16 [8,128,4096]; host unpacks (reshape + fp32 cast).
  - Total ~56 big DMAs (vs ~540 small in v2), spread over SP + POOL
    SWDGE queues.
"""

import sys

if "/opt/trn_rl_repo" not in sys.path:
    sys.path.insert(0, "/opt/trn_rl_repo")

import numpy as np

import concourse.bass as bass
import concourse.bacc as bacc
import concourse.mybir as mybir
import concourse.tile as tile

F32 = mybir.dt.float32
BF16 = mybir.dt.bfloat16

K, C, HEADS, S = 16, 64, 64, 4096
NG = 8
SC1 = 128          # phase-1 s-chunk (gram contraction tile)
NCH1 = S // SC1    # 32
SC2 = 512          # phase-2 s-chunk (one psum bank)
NCH2 = S // SC2    # 8

ALPHA = 1.6732632423543772
LAMBDA = 1.0507009873554805
LN_ALPHA = float(np.log(ALPHA))
LN_LAMBDA_ALPHA = float(np.log(LAMBDA * ALPHA))
GRAM_SCALE = float(LAMBDA * LAMBDA / 64.0)
LAM_BF16 = 1.046875  # bf16(lambda); residual uses this exactly


def _sigma(p):
    # storage head position p = 8g+u holds real head 8u+g
    return 8 * (p % 8) + (p // 8)


def _pi(u, m):
    # vg/tg row for (sub-head u, member m)
    return 64 * (m % 2) + 8 * u + (m // 2)


def host_constants(w_value, w_key, w_query, w_out, b_out):
    consts = {}
    # kq conv rhs [128, 256]: [(m2, c), (h, m2', o)] = delta(m2,m2')*W_h[o,c]
    wkq2 = np.zeros((128, 256), np.float32)
    for m2 in range(2):
        rows = slice(m2 * 64, (m2 + 1) * 64)
        wkq2[rows, m2 * 64: m2 * 64 + 64] = w_key.T
        wkq2[rows, 128 + m2 * 64: 128 + m2 * 64 + 64] = w_query.T
    consts["wkq2"] = wkq2

    # value conv lhsT [128,128]: [(m2,c),(a,p)] = delta(m2,a)*Wv[sigma(p),c]
    wv2 = np.zeros((128, 128), np.float32)
    for a in range(2):
        for p in range(64):
            wv2[a * 64:(a + 1) * 64, a * 64 + p] = w_value[_sigma(p), :]
    consts["wv2"] = wv2

    # out conv lhsT [128,128]: [(a,p'),(a',o)] = delta(a,a')*lam*Wout[o,sig(p')]
    wo2 = np.zeros((128, 128), np.float32)
    for a in range(2):
        for p in range(64):
            wo2[a * 64 + p, a * 64:(a + 1) * 64] = (
                LAMBDA * w_out[:, _sigma(p)])
    consts["wo2"] = wo2

    # gram psum: partition (8j+u), free (8i+u'); mask kills u != u'
    mask = np.zeros((128, 128), np.float32)
    for p in range(128):
        for f in range(128):
            if p % 8 == f % 8:
                mask[p, f] = 1.0
    consts["maskg"] = mask

    # P[(8m+u), pi(u,m)] = 1  (serves both sides)
    P = np.zeros((128, 128), np.float32)
    for u in range(8):
        for m in range(16):
            P[8 * m + u, _pi(u, m)] = 1.0
    consts["permP"] = P
    consts["permPp"] = P.copy()

    # dpat[pi(u,i), pi(u,j)] = delta(i,j) - 1/16
    D = np.zeros((128, 128), np.float32)
    for u in range(8):
        for i in range(16):
            for j in range(16):
                D[_pi(u, i), _pi(u, j)] = (1.0 if i == j else 0.0) - 1.0 / 16.0
    consts["dpat"] = D

    bo2 = np.concatenate([b_out, b_out]).astype(np.float32)
    consts["be_col"] = (bo2 + LN_LAMBDA_ALPHA).reshape(128, 1)
    consts["bra_col"] = (LAMBDA * bo2).reshape(128, 1)
    return consts


def make_in_maps(in_tensor, consts):
    """Per-core input dicts. in_tensor fp32 [8,16,64,64,64]."""
    import ml_dtypes
    in_maps = []
    for b in range(8):
        xb = np.ascontiguousarray(
            in_tensor[b].reshape(8, 128, S)).astype(ml_dtypes.bfloat16)
        m = {"xb": xb}
        m.update(consts)
        in_maps.append(m)
    return in_maps


def build_nc():
    nc = bacc.Bacc("TRN2", target_bir_lowering=False, debug=False)

    xb_d = nc.dram_tensor("xb", [8, 128, S], BF16, kind="ExternalInput")
    wkq2_d = nc.dram_tensor("wkq2", [128, 256], F32, kind="ExternalInput")
    wv2_d = nc.dram_tensor("wv2", [128, 128], F32, kind="ExternalInput")
    wo2_d = nc.dram_tensor("wo2", [128, 128], F32, kind="ExternalInput")
    mask_d = nc.dram_tensor("maskg", [128, 128], F32, kind="ExternalInput")
    permP_d = nc.dram_tensor("permP", [128, 128], F32, kind="ExternalInput")
    permPp_d = nc.dram_tensor("permPp", [128, 128], F32, kind="ExternalInput")
    dpat_d = nc.dram_tensor("dpat", [128, 128], F32, kind="ExternalInput")
    be_d = nc.dram_tensor("be_col", [128, 1], F32, kind="ExternalInput")
    bra_d = nc.dram_tensor("bra_col", [128, 1], F32, kind="ExternalInput")
    out_d = nc.dram_tensor("out", [8, 128, S], BF16, kind="ExternalOutput")

    vscr_d = nc.dram_tensor("vscr", [NG, 128, S], BF16)
    tscr_d = nc.dram_tensor("tscr", [8, 128, S], BF16)

    with tile.TileContext(nc) as tc:
        with (
            tc.tile_pool(name="persist", bufs=1) as persist,
            tc.tile_pool(name="xpool", bufs=1) as xpool,
            tc.tile_pool(name="sc8k", bufs=1) as sc8k,
            tc.tile_pool(name="outp", bufs=3) as outp,
        ):
            # ---- constants ----
            def load_cast(dram, shape, tag, dtype=BF16, eng=None):
                f = persist.tile(shape, F32, tag=tag + "f")
                nc.sync.dma_start(out=f, in_=dram[:, :])
                if dtype == F32:
                    return f
                b = persist.tile(shape, dtype, tag=tag)
                (eng or nc.gpsimd).tensor_copy(b, f)
                return b

            wkq_sb = load_cast(wkq2_d, [128, 256], "wkq")
            wv_sb = load_cast(wv2_d, [128, 128], "wv")
            wo_sb = load_cast(wo2_d, [128, 128], "wo")
            mask_sb = load_cast(mask_d, [128, 128], "mask", F32)
            permP_sb = load_cast(permP_d, [128, 128], "permP", F32)
            permPp_sb = load_cast(permPp_d, [128, 128], "permPp", F32)
            dpat_sb = load_cast(dpat_d, [128, 128], "dpat", F32)
            be_sb = persist.tile([128, 1], F32, tag="be")
            nc.sync.dma_start(out=be_sb, in_=be_d[:, :])
            bra_sb = persist.tile([128, 1], F32, tag="bra")
            nc.sync.dma_start(out=bra_sb, in_=bra_d[:, :])
            lna_sb = persist.tile([128, 1], F32, tag="lna")
            nc.vector.memset(lna_sb, LN_ALPHA)
            zero_sb = persist.tile([128, 1], F32, tag="zero")
            nc.vector.memset(zero_sb, 0.0)

            # ---- x tiles (already bf16 in DRAM) ----
            x_sb = []
            for t in range(8):
                xt = xpool.tile([128, S], BF16, tag=f"x{t}")
                nc.sync.dma_start(out=xt, in_=xb_d[t])
                x_sb.append(xt)

            # ---- value conv + scatter (overlaps phase 1) ----
            vstores = []
            # vscr[g] row = 64a + 8u + b holds member 2b+a, head 8u+g;
            # scatter of tile t: src partition (a, g, u) -> [b=t][a, g, u, s]
            vsc_view = vscr_d.rearrange(
                "g (a u b) s -> b a g u s", a=2, u=8, b=8)
            with tc.tile_pool(name="vps", bufs=3, space="PSUM") as vps:
                for t in range(8):
                    vt = sc8k.tile([128, S], BF16, tag=f"s{t}")
                    for ch in range(NCH2):
                        sl = slice(SC2 * ch, SC2 * (ch + 1))
                        ps = vps.tile([128, SC2], F32, tag="vps")
                        nc.tensor.matmul(ps, wv_sb, x_sb[t][:, sl],
                                         start=True, stop=True)
                        if ch % 2 == 0:
                            nc.vector.tensor_copy(vt[:, sl], ps)
                        else:
                            nc.scalar.copy(vt[:, sl], ps)
                    # src partitions (a half): p = 8g + u, g-major ->
                    # dst dims (g, u, s); one DMA per a (3-dim DMA AP limit)
                    for a in range(2):
                        si = nc.gpsimd.dma_start(
                            out=vsc_view[t, a],
                            in_=vt[64 * a: 64 * (a + 1), :])
                        vstores.append(si)

            # vg gathers (reuse v slots; dep on ALL v scatters)
            vg_sb = []
            for g in range(NG):
                vg = sc8k.tile([128, S], BF16, tag=f"s{g}")
                gi = nc.sync.dma_start(out=vg, in_=vscr_d[g])
                for si in vstores:
                    tile.add_dep_helper(gi.ins, si.ins, reason="vg after vsc")
                vg_sb.append(vg)

            # ---- phase 1: kq conv + gram ----
            bigB = []
            for g in range(NG):
                bigB_t = persist.tile([128, 128], BF16, tag=f"bigB{g}")
                bigB.append(bigB_t)

            with (
                tc.tile_pool(name="kqT", bufs=2) as kqTp,
                tc.tile_pool(name="p1sc", bufs=3) as p1sc,
                tc.tile_pool(name="kqps", bufs=2, space="PSUM") as kqps,
                tc.tile_pool(name="gramps", bufs=1, space="PSUM") as gramps,
            ):
                gram_ps = []
                for gb in range(2):
                    gram_t = gramps.tile([128, 512], F32, tag=f"gram{gb}")
                    gram_ps.append(gram_t)

                for sc in range(NCH1):
                    sl = slice(SC1 * sc, SC1 * (sc + 1))
                    # kqT col = h*1024 + m*64 + o, member m = 8q + 2tq + m2
                    kqT = kqTp.tile([128, 2048], BF16, tag="kqT")
                    for q in range(2):
                        # psum col = tq*256 + h*128 + z, z = m2*64 + o
                        ps = kqps.tile([128, 1024], F32, tag="kqps")
                        for tq in range(4):
                            t = q * 4 + tq
                            nc.tensor.matmul(
                                ps[:, 256 * tq: 256 * (tq + 1)],
                                x_sb[t][:, sl], wkq_sb,
                                start=True, stop=True)
                        # selu(t)/lam = min(alpha e^t - alpha, relu(t))
                        # e2/r col = h*512 + tq*128 + z (psum regrouped)
                        psv = ps.rearrange("p (tq h z) -> p h tq z",
                                           tq=4, h=2, z=128)
                        e2 = p1sc.tile([128, 1024], BF16, tag="e2")
                        e2v = e2.rearrange("p (h tq z) -> p h tq z",
                                           tq=4, h=2, z=128)
                        nc.scalar.activation(
                            out=e2v, in_=psv,
                            func=mybir.ActivationFunctionType.Exp,
                            bias=lna_sb[:, 0:1])
                        r = p1sc.tile([128, 1024], BF16, tag="r")
                        rv = r.rearrange("p (h tq z) -> p h tq z",
                                         tq=4, h=2, z=128)
                        if (sc + q) % 2 == 0:
                            nc.scalar.activation(
                                out=rv, in_=psv,
                                func=mybir.ActivationFunctionType.Relu,
                                bias=zero_sb[:, 0:1])
                        else:
                            nc.vector.tensor_scalar(
                                out=rv, in0=psv, scalar1=0.0, scalar2=None,
                                op0=mybir.AluOpType.max)
                        # kqT q-half: col = h*1024 + q*512 + tq*128 + z
                        kqo = kqT.rearrange("p (h q tq z) -> p h tq z q",
                                            h=2, q=2, tq=4, z=128)
                        nc.vector.scalar_tensor_tensor(
                            out=kqo[:, :, :, :, q],
                            in0=e2, scalar=ALPHA, in1=r,
                            op0=mybir.AluOpType.subtract,
                            op1=mybir.AluOpType.min)
                    # gram operand: col = h*1024 + f*8 + e, f = 8m+u, e = g
                    kqv = kqT.rearrange("p (h f e) -> p h e f",
                                        h=2, f=128, e=8)
                    for g in range(NG):
                        q_ap = kqv[:, 1, g, :]
                        k_ap = kqv[:, 0, g, :]
                        nc.tensor.matmul(
                            gram_ps[g // 4][:, 128 * (g % 4): 128 * (g % 4 + 1)],
                            q_ap, k_ap,
                            start=(sc == 0 and g % 4 == 0),
                            stop=(sc == NCH1 - 1 and g % 4 == 3))

                # ---- softmax + bigB ----
                for g in range(NG):
                    gp = gram_ps[g // 4][:, 128 * (g % 4): 128 * (g % 4 + 1)]
                    E = p1sc.tile([128, 128], F32, tag="E")
                    nc.scalar.activation(
                        out=E, in_=gp,
                        func=mybir.ActivationFunctionType.Exp,
                        bias=zero_sb[:, 0:1], scale=GRAM_SCALE)
                    Ssum = p1sc.tile([128, 8], F32, tag="Ssum")
                    nc.vector.tensor_reduce(
                        out=Ssum,
                        in_=E.rearrange("p (i u) -> p u i", i=16, u=8),
                        axis=mybir.AxisListType.X, op=mybir.AluOpType.add)
                    R = p1sc.tile([128, 8], F32, tag="R")
                    nc.vector.reciprocal(out=R, in_=Ssum)
                    Eu = E.rearrange("p (i u) -> p u i", i=16, u=8)
                    for u in range(8):
                        nc.vector.tensor_scalar(
                            out=Eu[:, u, :], in0=Eu[:, u, :],
                            scalar1=R[:, u: u + 1], scalar2=None,
                            op0=mybir.AluOpType.mult)
                    nc.vector.tensor_tensor(
                        out=E, in0=E, in1=mask_sb, op=mybir.AluOpType.mult)
                    c_ps = kqps.tile([128, 1024], F32, tag="kqps")
                    nc.tensor.matmul(c_ps[:, 0:128], E, permPp_sb,
                                     start=True, stop=True)
                    c_sb = p1sc.tile([128, 128], F32, tag="csb")
                    nc.scalar.copy(c_sb, c_ps[:, 0:128])
                    b_ps = kqps.tile([128, 1024], F32, tag="kqps")
                    nc.tensor.matmul(b_ps[:, 0:128], permP_sb, c_sb,
                                     start=True, stop=True)
                    nc.vector.scalar_tensor_tensor(
                        out=bigB[g], in0=b_ps[:, 0:128], scalar=1.0,
                        in1=dpat_sb,
                        op0=mybir.AluOpType.mult, op1=mybir.AluOpType.add)

            # ---- phase 2: mix -> t shuffle -> out conv + selu ----
            tsc_view = tscr_d.rearrange(
                "b (a gg u) s -> gg a u b s", a=2, gg=8, u=8)
            # mix chunk-outer; scatter each s-half as soon as all g's reach
            # it, so the out conv overlaps the back half of the mix.
            tstores = {0: [], 1: []}
            SH = S // 2
            with (
                tc.tile_pool(name="mps", bufs=4, space="PSUM") as mps,
                tc.tile_pool(name="ops", bufs=3, space="PSUM") as ops,
                tc.tile_pool(name="tgp", bufs=1) as tgp,
                tc.tile_pool(name="p2sc", bufs=2) as p2sc,
            ):
                NHC = NCH2 // 2  # chunks per half
                tg_sb = []
                for ch in range(NCH2):
                    h = ch // NHC
                    if ch % NHC == 0:
                        tg_sb = []
                        for g in range(NG):
                            tg = tgp.tile([128, SH], BF16, tag=f"tg{g}")
                            tg_sb.append(tg)
                    csl = slice(SC2 * (ch % NHC), SC2 * (ch % NHC + 1))
                    sl = slice(SC2 * ch, SC2 * (ch + 1))
                    for g in range(NG):
                        pm = mps.tile([128, SC2], F32, tag="mps")
                        nc.tensor.matmul(pm, bigB[g], vg_sb[g][:, sl],
                                         start=True, stop=True)
                        if g % 2 == 0:
                            nc.scalar.copy(tg_sb[g][:, csl], pm)
                        else:
                            nc.vector.tensor_copy(tg_sb[g][:, csl], pm)
                    if ch % NHC == NHC - 1:
                        hsl = slice(SH * h, SH * (h + 1))
                        for g in range(NG):
                            # src partitions (a half): p = 8u+b, u-major ->
                            # dst dims (u, b, s)
                            for a in range(2):
                                si = nc.gpsimd.dma_start(
                                    out=tsc_view[g, a][:, :, hsl],
                                    in_=tg_sb[g][64 * a: 64 * (a + 1), :])
                                tstores[h].append(si)

                tp_sb = []
                for t in range(8):
                    tp = sc8k.tile([128, S], BF16, tag=f"s{t}")
                    for h in range(2):
                        hsl = slice(SH * h, SH * (h + 1))
                        li = nc.sync.dma_start(out=tp[:, hsl],
                                               in_=tscr_d[t][:, hsl])
                        for si in tstores[h]:
                            tile.add_dep_helper(li.ins, si.ins,
                                                reason="tp after tsc")
                    tp_sb.append(tp)
                for t in range(8):
                    tp = tp_sb[t]
                    ot = outp.tile([128, S], BF16, tag="ot")
                    for ch in range(NCH2):
                        sl = slice(SC2 * ch, SC2 * (ch + 1))
                        po = ops.tile([128, SC2], F32, tag="ops")
                        nc.tensor.matmul(po, wo_sb, tp[:, sl],
                                         start=True, stop=True)
                        # y = lam~*x + lam*(Wout t);  selu(z)+lam*a =
                        # min(lam a e^z, relu(lam z + lam b) + lam a);
                        # the -lam*a is folded out on the host.
                        y = p2sc.tile([128, SC2], F32, tag="yf")
                        nc.vector.scalar_tensor_tensor(
                            out=y, in0=x_sb[t][:, sl], scalar=LAM_BF16,
                            in1=po,
                            op0=mybir.AluOpType.mult,
                            op1=mybir.AluOpType.add)
                        ef = p2sc.tile([128, SC2], BF16, tag="ef")
                        nc.scalar.activation(
                            out=ef, in_=y,
                            func=mybir.ActivationFunctionType.Exp,
                            bias=be_sb[:, 0:1], scale=float(1.0 / LAMBDA))
                        rf = p2sc.tile([128, SC2], BF16, tag="rf")
                        nc.scalar.activation(
                            out=rf, in_=y,
                            func=mybir.ActivationFunctionType.Relu,
                            bias=bra_sb[:, 0:1])
                        nc.vector.scalar_tensor_tensor(
                            out=ot[:, sl], in0=ef,
                            scalar=float(LAMBDA * ALPHA), in1=rf,
                            op0=mybir.AluOpType.subtract,
                            op1=mybir.AluOpType.min)
                    nc.sync.dma_start(out=out_d[t], in_=ot)
    nc.compile()
    return nc


_NC_CACHE = None


def _get_nc():
    global _NC_CACHE
    if _NC_CACHE is None:
        _NC_CACHE = build_nc()
    return _NC_CACHE


def kernel(in_tensor, w_value, w_key, w_query, w_out, b_out, **_ignored):
    in_tensor = np.asarray(in_tensor, dtype=np.float32)
    consts = host_constants(
        np.asarray(w_value, dtype=np.float32),
        np.asarray(w_key, dtype=np.float32),
        np.asarray(w_query, dtype=np.float32),
        np.asarray(w_out, dtype=np.float32),
        np.asarray(b_out, dtype=np.float32))
    assert in_tensor.shape[0] == 8
    in_maps = make_in_maps(in_tensor, consts)

    nc = _get_nc()
    from concourse.bass_utils import run_bass_kernel_spmd
    res = run_bass_kernel_spmd(nc, in_maps, core_ids=list(range(8)))
    outs = [np.asarray(res.results[b]["out"]).astype(np.float32)
            .reshape(K, C, 64, 64) for b in range(8)]
    return np.stack(outs, axis=0)


if __name__ == "__main__":
    build_nc()
    print("built ok")


# revision 42
# speedup vs baseline: 3.1462x; 1.0315x over previous
"""Trainium2 Bass kernel for nn_BaseTransformer (ensemble member-attention).

Sharding: data-parallel over batch B=8 across 8 NeuronCores (1 batch each).

v3 design (DMA-dispatch-count diet vs v2 baseline):
  - Host uploads x pre-packed as bf16 pair tiles xb[t] = members (2t, 2t+1)
    rows (m2*64 + c); pure reshape + cast on host. Device never casts x.
  - Value conv first (block-diag WvT, full 128-contract), v tiles scattered
    to DRAM vscr in group-major layout with 4 KB-run descriptors; the
    gathers ride during phase 1 (kq conv + gram), fully hidden.
  - kq conv: x chunk stationary, streams block-diag [128,256] wkq ->
    psum col order (tq, m2, h, o) == kqT col m*128 + h*64 + o for member
    m = 2t + m2, so selu (exp/relu/stt) writes are plain contiguous and
    the gram operands are clean 3-dim APs [s | m:128 | u:8] @ h*64+g.
  - softmax -> bigB via host permutation matmuls (P, P', dpat) with
    pi(u, m) = 64*(m%2) + 8u + m//2.
  - mix per head-group g (bigB stationary), tg scattered to tscr
    (pair-major), tp gathered, out conv = wo2 matmul + residual
    lam~*I matmul (lam~ = bf16(lambda)), selu via exp/ts/stt, out
    written as bf16 [8,128,4096]; host unpacks (reshape + fp32 cast).
  - Total ~56 big DMAs (vs ~540 small in v2), spread over SP + POOL
    SWDGE queues.
"""

import sys

if "/opt/trn_rl_repo" not in sys.path:
    sys.path.insert(0, "/opt/trn_rl_repo")

import numpy as np

import concourse.bass as bass
import concourse.bacc as bacc
import concourse.mybir as mybir
import concourse.tile as tile

F32 = mybir.dt.float32
BF16 = mybir.dt.bfloat16

K, C, HEADS, S = 16, 64, 64, 4096
NG = 8
SC1 = 128          # phase-1 s-chunk (gram contraction tile)
NCH1 = S // SC1    # 32
SC2 = 512          # phase-2 s-chunk (one psum bank)
NCH2 = S // SC2    # 8

ALPHA = 1.6732632423543772
LAMBDA = 1.0507009873554805
LN_ALPHA = float(np.log(ALPHA))
LN_LAMBDA_ALPHA = float(np.log(LAMBDA * ALPHA))
GRAM_SCALE = float(LAMBDA * LAMBDA / 64.0)
LAM_BF16 = 1.046875  # bf16(lambda); residual uses this exactly


def _sigma(p):
    # storage head position p = 8g+u holds real head 8u+g
    return 8 * (p % 8) + (p // 8)


def _pi(u, m):
    # vg/tg row for (sub-head u, member m)
    return 64 * (m % 2) + 8 * u + (m // 2)


def host_constants(w_value, w_key, w_query, w_out, b_out):
    consts = {}
    # kq conv rhs [128, 256]: [(m2, c), (h, m2', o)] = delta(m2,m2')*W_h[o,c]
    wkq2 = np.zeros((128, 256), np.float32)
    for m2 in range(2):
        rows = slice(m2 * 64, (m2 + 1) * 64)
        wkq2[rows, m2 * 64: m2 * 64 + 64] = w_key.T
        wkq2[rows, 128 + m2 * 64: 128 + m2 * 64 + 64] = w_query.T
    consts["wkq2"] = wkq2

    # value conv lhsT [128,128]: [(m2,c),(a,p)] = delta(m2,a)*Wv[sigma(p),c]
    wv2 = np.zeros((128, 128), np.float32)
    for a in range(2):
        for p in range(64):
            wv2[a * 64:(a + 1) * 64, a * 64 + p] = w_value[_sigma(p), :]
    consts["wv2"] = wv2

    # out conv lhsT [128,128]: [(a,p'),(a',o)] = delta(a,a')*lam*Wout[o,sig(p')]
    wo2 = np.zeros((128, 128), np.float32)
    for a in range(2):
        for p in range(64):
            wo2[a * 64 + p, a * 64:(a + 1) * 64] = (
                LAMBDA * w_out[:, _sigma(p)])
    consts["wo2"] = wo2

    # residual lhsT: lam~ * I (entries exactly representable in bf16)
    consts["resI"] = (LAM_BF16 * np.eye(128)).astype(np.float32)

    # gram psum: partition (8j+u), free (8i+u'); mask kills u != u'
    mask = np.zeros((128, 128), np.float32)
    for p in range(128):
        for f in range(128):
            if p % 8 == f % 8:
                mask[p, f] = 1.0
    consts["maskg"] = mask

    # P[(8m+u), pi(u,m)] = 1  (serves both sides)
    P = np.zeros((128, 128), np.float32)
    for u in range(8):
        for m in range(16):
            P[8 * m + u, _pi(u, m)] = 1.0
    consts["permP"] = P
    consts["permPp"] = P.copy()

    # dpat[pi(u,i), pi(u,j)] = delta(i,j) - 1/16
    D = np.zeros((128, 128), np.float32)
    for u in range(8):
        for i in range(16):
            for j in range(16):
                D[_pi(u, i), _pi(u, j)] = (1.0 if i == j else 0.0) - 1.0 / 16.0
    consts["dpat"] = D

    bo2 = np.concatenate([b_out, b_out]).astype(np.float32)
    consts["be_col"] = (bo2 + LN_LAMBDA_ALPHA).reshape(128, 1)
    consts["bra_col"] = (LAMBDA * bo2).reshape(128, 1)
    return consts


def make_in_maps(in_tensor, consts):
    """Per-core input dicts. in_tensor fp32 [8,16,64,64,64]."""
    import ml_dtypes
    in_maps = []
    for b in range(8):
        xb = np.ascontiguousarray(
            in_tensor[b].reshape(8, 128, S)).astype(ml_dtypes.bfloat16)
        m = {"xb": xb}
        m.update(consts)
        in_maps.append(m)
    return in_maps


def build_nc():
    nc = bacc.Bacc("TRN2", target_bir_lowering=False, debug=False)

    xb_d = nc.dram_tensor("xb", [8, 128, S], BF16, kind="ExternalInput")
    wkq2_d = nc.dram_tensor("wkq2", [128, 256], F32, kind="ExternalInput")
    wv2_d = nc.dram_tensor("wv2", [128, 128], F32, kind="ExternalInput")
    wo2_d = nc.dram_tensor("wo2", [128, 128], F32, kind="ExternalInput")
    resI_d = nc.dram_tensor("resI", [128, 128], F32, kind="ExternalInput")
    mask_d = nc.dram_tensor("maskg", [128, 128], F32, kind="ExternalInput")
    permP_d = nc.dram_tensor("permP", [128, 128], F32, kind="ExternalInput")
    permPp_d = nc.dram_tensor("permPp", [128, 128], F32, kind="ExternalInput")
    dpat_d = nc.dram_tensor("dpat", [128, 128], F32, kind="ExternalInput")
    be_d = nc.dram_tensor("be_col", [128, 1], F32, kind="ExternalInput")
    bra_d = nc.dram_tensor("bra_col", [128, 1], F32, kind="ExternalInput")
    out_d = nc.dram_tensor("out", [8, 128, S], BF16, kind="ExternalOutput")

    vscr_d = nc.dram_tensor("vscr", [NG, 128, S], BF16)
    tscr_d = nc.dram_tensor("tscr", [8, 128, S], BF16)

    with tile.TileContext(nc) as tc:
        with (
            tc.tile_pool(name="persist", bufs=1) as persist,
            tc.tile_pool(name="xpool", bufs=1) as xpool,
            tc.tile_pool(name="sc8k", bufs=1) as sc8k,
            tc.tile_pool(name="outp", bufs=3) as outp,
        ):
            # ---- constants ----
            def load_cast(dram, shape, tag, dtype=BF16, eng=None):
                f = persist.tile(shape, F32, tag=tag + "f")
                nc.sync.dma_start(out=f, in_=dram[:, :])
                if dtype == F32:
                    return f
                b = persist.tile(shape, dtype, tag=tag)
                (eng or nc.gpsimd).tensor_copy(b, f)
                return b

            wkq_sb = load_cast(wkq2_d, [128, 256], "wkq")
            wv_sb = load_cast(wv2_d, [128, 128], "wv")
            wo_sb = load_cast(wo2_d, [128, 128], "wo")
            resI_sb = load_cast(resI_d, [128, 128], "resI")
            mask_sb = load_cast(mask_d, [128, 128], "mask", F32)
            permP_sb = load_cast(permP_d, [128, 128], "permP", F32)
            permPp_sb = load_cast(permPp_d, [128, 128], "permPp", F32)
            dpat_sb = load_cast(dpat_d, [128, 128], "dpat", F32)
            be_sb = persist.tile([128, 1], F32, tag="be")
            nc.sync.dma_start(out=be_sb, in_=be_d[:, :])
            bra_sb = persist.tile([128, 1], F32, tag="bra")
            nc.sync.dma_start(out=bra_sb, in_=bra_d[:, :])
            lna_sb = persist.tile([128, 1], F32, tag="lna")
            nc.vector.memset(lna_sb, LN_ALPHA)
            zero_sb = persist.tile([128, 1], F32, tag="zero")
            nc.vector.memset(zero_sb, 0.0)

            # ---- x tiles (already bf16 in DRAM) ----
            x_sb = []
            for t in range(8):
                xt = xpool.tile([128, S], BF16, tag=f"x{t}")
                nc.sync.dma_start(out=xt, in_=xb_d[t])
                x_sb.append(xt)

            # ---- value conv + scatter (overlaps phase 1) ----
            vstores = []
            # vscr[g] row = 64a + 8u + b holds member 2b+a, head 8u+g;
            # scatter of tile t: src partition (a, g, u) -> [b=t][a, g, u, s]
            vsc_view = vscr_d.rearrange(
                "g (a u b) s -> b a g u s", a=2, u=8, b=8)
            with tc.tile_pool(name="vps", bufs=3, space="PSUM") as vps:
                for t in range(8):
                    vt = sc8k.tile([128, S], BF16, tag=f"s{t}")
                    for ch in range(NCH2):
                        sl = slice(SC2 * ch, SC2 * (ch + 1))
                        ps = vps.tile([128, SC2], F32, tag="vps")
                        nc.tensor.matmul(ps, wv_sb, x_sb[t][:, sl],
                                         start=True, stop=True)
                        if ch % 2 == 0:
                            nc.vector.tensor_copy(vt[:, sl], ps)
                        else:
                            nc.scalar.copy(vt[:, sl], ps)
                    # src partitions (a half): p = 8g + u, g-major ->
                    # dst dims (g, u, s); one DMA per a (3-dim DMA AP limit)
                    for a in range(2):
                        si = nc.gpsimd.dma_start(
                            out=vsc_view[t, a],
                            in_=vt[64 * a: 64 * (a + 1), :])
                        vstores.append(si)

            # vg gathers (reuse v slots; dep on ALL v scatters)
            vg_sb = []
            for g in range(NG):
                vg = sc8k.tile([128, S], BF16, tag=f"s{g}")
                gi = nc.sync.dma_start(out=vg, in_=vscr_d[g])
                for si in vstores:
                    tile.add_dep_helper(gi.ins, si.ins, reason="vg after vsc")
                vg_sb.append(vg)

            # ---- phase 1: kq conv + gram ----
            bigB = []
            for g in range(NG):
                bigB_t = persist.tile([128, 128], BF16, tag=f"bigB{g}")
                bigB.append(bigB_t)

            with (
                tc.tile_pool(name="kqT", bufs=2) as kqTp,
                tc.tile_pool(name="p1sc", bufs=3) as p1sc,
                tc.tile_pool(name="kqps", bufs=2, space="PSUM") as kqps,
                tc.tile_pool(name="gramps", bufs=1, space="PSUM") as gramps,
            ):
                gram_ps = []
                for gb in range(2):
                    gram_t = gramps.tile([128, 512], F32, tag=f"gram{gb}")
                    gram_ps.append(gram_t)

                for sc in range(NCH1):
                    sl = slice(SC1 * sc, SC1 * (sc + 1))
                    # kqT col = h*1024 + m*64 + o, member m = 8q + 2tq + m2
                    kqT = kqTp.tile([128, 2048], BF16, tag="kqT")
                    for q in range(2):
                        # psum col = tq*256 + h*128 + z, z = m2*64 + o
                        ps = kqps.tile([128, 1024], F32, tag="kqps")
                        for tq in range(4):
                            t = q * 4 + tq
                            nc.tensor.matmul(
                                ps[:, 256 * tq: 256 * (tq + 1)],
                                x_sb[t][:, sl], wkq_sb,
                                start=True, stop=True)
                        # selu(t)/lam = min(alpha e^t - alpha, relu(t))
                        # e2/r col = h*512 + tq*128 + z (psum regrouped)
                        psv = ps.rearrange("p (tq h z) -> p h tq z",
                                           tq=4, h=2, z=128)
                        e2 = p1sc.tile([128, 1024], BF16, tag="e2")
                        e2v = e2.rearrange("p (h tq z) -> p h tq z",
                                           tq=4, h=2, z=128)
                        nc.scalar.activation(
                            out=e2v, in_=psv,
                            func=mybir.ActivationFunctionType.Exp,
                            bias=lna_sb[:, 0:1])
                        r = p1sc.tile([128, 1024], BF16, tag="r")
                        rv = r.rearrange("p (h tq z) -> p h tq z",
                                         tq=4, h=2, z=128)
                        if (sc + q) % 2 == 0:
                            nc.scalar.activation(
                                out=rv, in_=psv,
                                func=mybir.ActivationFunctionType.Relu,
                                bias=zero_sb[:, 0:1])
                        else:
                            nc.vector.tensor_scalar(
                                out=rv, in0=psv, scalar1=0.0, scalar2=None,
                                op0=mybir.AluOpType.max)
                        # kqT q-half: col = h*1024 + q*512 + tq*128 + z
                        kqo = kqT.rearrange("p (h q tq z) -> p h tq z q",
                                            h=2, q=2, tq=4, z=128)
                        nc.vector.scalar_tensor_tensor(
                            out=kqo[:, :, :, :, q],
                            in0=e2, scalar=ALPHA, in1=r,
                            op0=mybir.AluOpType.subtract,
                            op1=mybir.AluOpType.min)
                    # gram operand: col = h*1024 + f*8 + e, f = 8m+u, e = g
                    kqv = kqT.rearrange("p (h f e) -> p h e f",
                                        h=2, f=128, e=8)
                    for g in range(NG):
                        q_ap = kqv[:, 1, g, :]
                        k_ap = kqv[:, 0, g, :]
                        nc.tensor.matmul(
                            gram_ps[g // 4][:, 128 * (g % 4): 128 * (g % 4 + 1)],
                            q_ap, k_ap,
                            start=(sc == 0 and g % 4 == 0),
                            stop=(sc == NCH1 - 1 and g % 4 == 3))

                # ---- softmax + bigB ----
                for g in range(NG):
                    gp = gram_ps[g // 4][:, 128 * (g % 4): 128 * (g % 4 + 1)]
                    E = p1sc.tile([128, 128], F32, tag="E")
                    nc.scalar.activation(
                        out=E, in_=gp,
                        func=mybir.ActivationFunctionType.Exp,
                        bias=zero_sb[:, 0:1], scale=GRAM_SCALE)
                    Ssum = p1sc.tile([128, 8], F32, tag="Ssum")
                    nc.vector.tensor_reduce(
                        out=Ssum,
                        in_=E.rearrange("p (i u) -> p u i", i=16, u=8),
                        axis=mybir.AxisListType.X, op=mybir.AluOpType.add)
                    R = p1sc.tile([128, 8], F32, tag="R")
                    nc.vector.reciprocal(out=R, in_=Ssum)
                    Eu = E.rearrange("p (i u) -> p u i", i=16, u=8)
                    for u in range(8):
                        nc.vector.tensor_scalar(
                            out=Eu[:, u, :], in0=Eu[:, u, :],
                            scalar1=R[:, u: u + 1], scalar2=None,
                            op0=mybir.AluOpType.mult)
                    nc.vector.tensor_tensor(
                        out=E, in0=E, in1=mask_sb, op=mybir.AluOpType.mult)
                    c_ps = kqps.tile([128, 1024], F32, tag="kqps")
                    nc.tensor.matmul(c_ps[:, 0:128], E, permPp_sb,
                                     start=True, stop=True)
                    c_sb = p1sc.tile([128, 128], F32, tag="csb")
                    nc.scalar.copy(c_sb, c_ps[:, 0:128])
                    b_ps = kqps.tile([128, 1024], F32, tag="kqps")
                    nc.tensor.matmul(b_ps[:, 0:128], permP_sb, c_sb,
                                     start=True, stop=True)
                    nc.vector.scalar_tensor_tensor(
                        out=bigB[g], in0=b_ps[:, 0:128], scalar=1.0,
                        in1=dpat_sb,
                        op0=mybir.AluOpType.mult, op1=mybir.AluOpType.add)

            # ---- phase 2: mix -> t shuffle -> out conv + selu ----
            tsc_view = tscr_d.rearrange(
                "b (a gg u) s -> gg a u b s", a=2, gg=8, u=8)
            # mix chunk-outer; scatter each s-half as soon as all g's reach
            # it, so the out conv overlaps the back half of the mix.
            tstores = {0: [], 1: []}
            SH = S // 2
            with (
                tc.tile_pool(name="mps", bufs=4, space="PSUM") as mps,
                tc.tile_pool(name="ops", bufs=3, space="PSUM") as ops,
                tc.tile_pool(name="tgp", bufs=1) as tgp,
                tc.tile_pool(name="p2sc", bufs=2) as p2sc,
            ):
                NHC = NCH2 // 2  # chunks per half
                tg_sb = []
                for ch in range(NCH2):
                    h = ch // NHC
                    if ch % NHC == 0:
                        tg_sb = []
                        for g in range(NG):
                            tg = tgp.tile([128, SH], BF16, tag=f"tg{g}")
                            tg_sb.append(tg)
                    csl = slice(SC2 * (ch % NHC), SC2 * (ch % NHC + 1))
                    sl = slice(SC2 * ch, SC2 * (ch + 1))
                    for g in range(NG):
                        pm = mps.tile([128, SC2], F32, tag="mps")
                        nc.tensor.matmul(pm, bigB[g], vg_sb[g][:, sl],
                                         start=True, stop=True)
                        if g % 2 == 0:
                            nc.scalar.copy(tg_sb[g][:, csl], pm)
                        else:
                            nc.vector.tensor_copy(tg_sb[g][:, csl], pm)
                    if ch % NHC == NHC - 1:
                        hsl = slice(SH * h, SH * (h + 1))
                        for g in range(NG):
                            # src partitions (a half): p = 8u+b, u-major ->
                            # dst dims (u, b, s)
                            for a in range(2):
                                si = nc.gpsimd.dma_start(
                                    out=tsc_view[g, a][:, :, hsl],
                                    in_=tg_sb[g][64 * a: 64 * (a + 1), :])
                                tstores[h].append(si)

                tp_sb = []
                for t in range(8):
                    tp = sc8k.tile([128, S], BF16, tag=f"s{t}")
                    for h in range(2):
                        hsl = slice(SH * h, SH * (h + 1))
                        li = nc.sync.dma_start(out=tp[:, hsl],
                                               in_=tscr_d[t][:, hsl])
                        for si in tstores[h]:
                            tile.add_dep_helper(li.ins, si.ins,
                                                reason="tp after tsc")
                    tp_sb.append(tp)
                for t in range(8):
                    tp = tp_sb[t]
                    ot = outp.tile([128, S], BF16, tag="ot")
                    for ch in range(NCH2):
                        sl = slice(SC2 * ch, SC2 * (ch + 1))
                        po = ops.tile([128, SC2], F32, tag="ops")
                        nc.tensor.matmul(po, wo_sb, tp[:, sl],
                                         start=True, stop=False)
                        nc.tensor.matmul(po, resI_sb, x_sb[t][:, sl],
                                         start=False, stop=True)
                        # po ~ lam*(x + Wout t); selu:
                        # out = min(lam a e^z - lam a, relu(lam z + lam b))
                        ef = p2sc.tile([128, SC2], BF16, tag="ef")
                        nc.scalar.activation(
                            out=ef, in_=po,
                            func=mybir.ActivationFunctionType.Exp,
                            bias=be_sb[:, 0:1], scale=float(1.0 / LAMBDA))
                        rf = p2sc.tile([128, SC2], BF16, tag="rf")
                        if ch % 2 == 0:
                            nc.scalar.activation(
                                out=rf, in_=po,
                                func=mybir.ActivationFunctionType.Relu,
                                bias=bra_sb[:, 0:1])
                        else:
                            nc.vector.tensor_scalar(
                                out=rf, in0=po, scalar1=bra_sb[:, 0:1],
                                scalar2=0.0,
                                op0=mybir.AluOpType.add,
                                op1=mybir.AluOpType.max)
                        nc.vector.scalar_tensor_tensor(
                            out=ot[:, sl], in0=ef,
                            scalar=float(LAMBDA * ALPHA), in1=rf,
                            op0=mybir.AluOpType.subtract,
                            op1=mybir.AluOpType.min)
                    nc.sync.dma_start(out=out_d[t], in_=ot)
    nc.compile()
    return nc


_NC_CACHE = None


def _get_nc():
    global _NC_CACHE
    if _NC_CACHE is None:
        _NC_CACHE = build_nc()
    return _NC_CACHE


def kernel(in_tensor, w_value, w_key, w_query, w_out, b_out, **_ignored):
    in_tensor = np.asarray(in_tensor, dtype=np.float32)
    consts = host_constants(
        np.asarray(w_value, dtype=np.float32),
        np.asarray(w_key, dtype=np.float32),
        np.asarray(w_query, dtype=np.float32),
        np.asarray(w_out, dtype=np.float32),
        np.asarray(b_out, dtype=np.float32))
    assert in_tensor.shape[0] == 8
    in_maps = make_in_maps(in_tensor, consts)

    nc = _get_nc()
    from concourse.bass_utils import run_bass_kernel_spmd
    res = run_bass_kernel_spmd(nc, in_maps, core_ids=list(range(8)))
    outs = [np.asarray(res.results[b]["out"]).astype(np.float32)
            .reshape(K, C, 64, 64) for b in range(8)]
    return np.stack(outs, axis=0)


if __name__ == "__main__":
    build_nc()
    print("built ok")
